# revision 1
# baseline (speedup 1.0000x reference)
"""Trainium2 Bass kernel for nn_HRRAdaptedAttention (B=2, S=8192, D=1024).

out = output + gate * irfft(cumsum_s(rfft(k)*rfft(v)) * conj(rfft(q))),
q/k/v = hidden @ W.T + b.

Single merged launch per core (chunk = 2048 positions, (batch, quarter)
per core). The rfft/irfft are folded into the projection weights on the
host; the nyquist row is packed into the (all-zero) sin(f=0) column of
each S-matrix, with 1-row DVE fixups where the complex-multiply formula
would mix the DC and nyquist rows. Projection matmuls run as fp8e4m3
hi+lo splits in DoubleRow perf mode (2 contraction slices/instr at 0.5
cyc/row); the cross-chunk cumsum carry is resolved in-kernel with an
AllGather of per-chunk totals (hidden under the fq matmuls), so there
is no second launch and no kv/ht DRAM round trip.
"""

import numpy as np
import ml_dtypes

B, S, D = 2, 8192, 1024
NCORES = 8
CHUNK = 2048
PANEL = 512
NPANEL = CHUNK // PANEL
FT = 4                    # 128-row freq tiles f=0..511 (nyq packed in im row 0)
NDP = 8                   # 128-row contraction slices of D
NPAIR = NDP // 2

F8NP = ml_dtypes.float8_e4m3
BF16NP = ml_dtypes.bfloat16

_cache = {}


def _split8(x):
    hi = x.astype(F8NP)
    lo = (x - hi.astype(np.float32)).astype(F8NP)
    return hi, lo


def _pack_w(M):
    """[1024, 512] f32 -> [128, dp(8), hl(2), 512] fp8 -> [128, 8192]."""
    hi, lo = _split8(np.asarray(M, np.float32))
    out = np.empty((128, NDP, 2, 512), F8NP)
    out[:, :, 0, :] = hi.reshape(NDP, 128, 512).transpose(1, 0, 2)
    out[:, :, 1, :] = lo.reshape(NDP, 128, 512).transpose(1, 0, 2)
    return out.reshape(128, -1)


def _pack_ht(ht):
    """[1024, 2048] f32 -> [128, pan(4), dp(8), hl(2), pos(512)] fp8."""
    hi, lo = _split8(ht)
    out = np.empty((128, NPANEL, NDP, 2, PANEL), F8NP)
    h4 = hi.reshape(NDP, 128, NPANEL, PANEL)
    l4 = lo.reshape(NDP, 128, NPANEL, PANEL)
    out[:, :, :, 0, :] = h4.transpose(1, 2, 0, 3)
    out[:, :, :, 1, :] = l4.transpose(1, 2, 0, 3)
    return out.reshape(128, -1)


def _host_constants(Wq, bq, Wk, bk, Wv, bv, gate):
    d = np.arange(D, dtype=np.float64)
    f = np.arange(513, dtype=np.float64)
    ang = 2.0 * np.pi * np.outer(d, f) / D
    C = np.cos(ang)
    Sm = -np.sin(ang)

    def fold(W, sign_s=1.0):
        Wt = W.T.astype(np.float64)
        MC = Wt @ C
        MS = sign_s * (Wt @ Sm)
        MS[:, 0] = MC[:, 512]          # nyquist packed into sin(f=0) col
        return MC[:, :512].astype(np.float32), MS[:, :512].astype(np.float32)

    MkC, MkS = fold(Wk)
    MvC, MvS = fold(Wv)
    MqC, MqS = fold(Wq, sign_s=-1.0)   # conj(fq) folded; nyq col has no sign
    # Z = mem*conj(fq) is ~1e6-scale, far outside fp8 range. The fq
    # PSUM->SBUF copy applies 2^-19 (activation scale) so Z comes out
    # pre-scaled; A/B carry the inverse. Pure exponent shift.
    ZSC = 2.0 ** -19

    g = float(np.asarray(gate).reshape(-1)[0])
    w = np.full(513, 2.0)
    w[0] = 1.0
    w[512] = 1.0
    scale = (w * g / (D * ZSC))[:, None]
    A = (scale * C.T).astype(np.float32)       # [513, D]
    Bm = (scale * Sm.T).astype(np.float32)
    Bout = Bm[:512].copy()
    Bout[0] = A[512]                            # nyquist row in B row 0
    Aout = A[:512]

    def foldb(b, sign_s=1.0):
        b64 = np.asarray(b, np.float64)
        bc = b64 @ C
        bs = sign_s * (b64 @ Sm)
        bs[0] = bc[512]
        return bc[:512].astype(np.float32), bs[:512].astype(np.float32)

    bkC, bkS = foldb(bk)
    bvC, bvS = foldb(bv)
    bqC, bqS = foldb(bq, sign_s=-1.0)
    bqC *= ZSC
    bqS *= ZSC
    biasc = np.stack([bkC, bkS, bvC, bvS, bqC, bqS])   # [6, 512]

    wall = np.concatenate(
        [_pack_w(M) for M in (MkC, MkS, MvC, MvS, MqC, MqS)], axis=1)
    ab = np.concatenate([Aout, Bout], axis=0).astype(BF16NP)  # [1024, 1024]
    return dict(wall=wall, ab=ab, biasc=biasc)


_WAIT_EXEMPT = {
    "InstNoOp", "InstEventSemaphore", "InstUnconditionalBranch",
    "InstRegisterMove", "InstCall", "InstISA",
}


def _legalize_waits(nc, max_waits=1):
    """TRN2 instruction structs hold one sync-wait command; move extra waits
    onto same-engine nops inserted just before the instruction."""
    import bass_rust
    import concourse.mybir as mybir
    ctr = 0
    for fn in nc.m.functions:
        for blk in fn.blocks:
            new = []
            for inst in blk.instructions:
                if (type(inst).__name__ not in _WAIT_EXEMPT
                        and inst.sync_info is not None):
                    waits = list(inst.sync_info.on_wait)
                    if len(waits) > max_waits:
                        for w in waits[:-max_waits]:
                            nop = mybir.InstNoOp(
                                name=f"I-lglnop-{ctr}", ins=[], outs=[])
                            ctr += 1
                            nop.engine = inst.engine
                            nop.sync_info = bass_rust.SyncInfo(
                                on_wait=[w], on_update=[])
                            new.append(nop)
                        inst.sync_info = bass_rust.SyncInfo(
                            on_wait=waits[-max_waits:],
                            on_update=inst.sync_info.on_update)
                new.append(inst)
            blk.instructions = new


def _build(has_bias):
    import concourse.bass as bass
    import concourse.mybir as mybir
    import concourse.tile as tile
    F32, F8, BF16 = mybir.dt.float32, mybir.dt.float8e4, mybir.dt.bfloat16
    AT = mybir.AluOpType
    DR = mybir.MatmulPerfMode.DoubleRow

    nc = bass.Bass("TRN2", target_bir_lowering=False, debug=False,
                   num_devices=NCORES)
    htp_d = nc.dram_tensor("htp", [128, NPANEL * 8192], F8,
                           kind="ExternalInput")
    w_d = nc.dram_tensor("wall", [128, 6 * 8192], F8, kind="ExternalInput")
    ab_d = nc.dram_tensor("ab", [1024, 1024], BF16, kind="ExternalInput")
    outp_d = nc.dram_tensor("outp", [CHUNK, D], BF16, kind="ExternalInput")
    mask_d = nc.dram_tensor("mask", [128, 64], F32, kind="ExternalInput")
    if has_bias:
        biasc_d = nc.dram_tensor("biasc", [128, 24], F32,
                                 kind="ExternalInput")
    res_d = nc.dram_tensor("res", [CHUNK, D], F32, kind="ExternalOutput")
    cc_in = nc.dram_tensor("cc_in", [128, 8], F32)
    cc_out = nc.dram_tensor("cc_out", [NCORES * 128, 8], F32)

    with tile.TileContext(nc) as tc:
        with (
            tc.tile_pool(name="wpool", bufs=1) as wp,
            tc.tile_pool(name="htpool", bufs=1) as hp,
            tc.tile_pool(name="const", bufs=1) as cp,
            tc.tile_pool(name="fkv", bufs=2) as fkp,
            tc.tile_pool(name="fq3", bufs=3) as fqp,
            tc.tile_pool(name="tt", bufs=4) as ttp,
            tc.tile_pool(name="kv", bufs=2) as kvp,
            tc.tile_pool(name="mem", bufs=1) as memp,
            tc.tile_pool(name="z", bufs=2) as zp,
            tc.tile_pool(name="io", bufs=2) as iop,
            tc.tile_pool(name="ps", bufs=2, space="PSUM") as psp,
        ):
            # DMA order is tuned so the PE can start ~7us in: hi halves of
            # the k/v weights and panel-0 ht first, then the lo halves.
            w = [wp.tile([128, 8192], F8, tag=f"w{m}", name=f"w{m}") for m in range(6)]
            htp = [hp.tile([128, 8192], F8, tag=f"ht{p}", name=f"ht{p}")
                   for p in range(NPANEL)]
            wvd = w_d.ap().rearrange("p (m dp hl f) -> p m dp hl f",
                                     m=6, dp=NDP, hl=2)
            htd = htp_d.ap().rearrange("p (pan dp hl x) -> p pan dp hl x",
                                       pan=NPANEL, dp=NDP, hl=2)
            wvs = [w[m][:].rearrange("p (dp hl f) -> p dp hl f",
                                     dp=NDP, hl=2) for m in range(6)]
            hts = [htp[p][:].rearrange("p (dp hl x) -> p dp hl x",
                                       dp=NDP, hl=2) for p in range(NPANEL)]

            def load_w(m, hl):
                nc.sync.dma_start(wvs[m][:, :, hl, :], wvd[:, m, :, hl, :])

            def load_ht(p, hl):
                nc.sync.dma_start(hts[p][:, :, hl, :], htd[:, p, :, hl, :])

            load_ht(0, 0)
            load_w(0, 0)
            load_w(0, 1)
            load_ht(0, 1)
            for m in range(1, 4):
                load_w(m, 0)
                load_w(m, 1)
            load_w(4, 0)
            load_w(5, 0)
            load_w(4, 1)
            load_w(5, 1)
            for p in range(1, NPANEL):
                load_ht(p, 0)
                load_ht(p, 1)
            ab = [wp.tile([128, 1024], BF16, tag=f"ab{i}", name=f"ab{i}")
                  for i in range(8)]
            for i in range(8):
                nc.sync.dma_start(ab[i][:], ab_d.ap()[i * 128:(i + 1) * 128, :])
            mask = cp.tile([128, 64], F32, tag="mask", name="mask")
            nc.sync.dma_start(mask[:], mask_d.ap())
            if has_bias:
                biasc = cp.tile([128, 24], F32, tag="biasc", name="biasc")
                nc.sync.dma_start(biasc[:], biasc_d.ap())

            wv, htv = wvs, hts

            def fwd_matmuls(pt, m, pan, ft):
                ii = 0
                # hh first (hi weights + hi data), then lh (lo weights),
                # then hl (lo data) — matches the DMA arrival order.
                for (whl, xhl) in ((0, 0), (1, 0), (0, 1)):
                    for a in range(NPAIR):
                        nc.tensor.matmul(
                            pt[:],
                            wv[m][:, 2 * a:2 * a + 2, whl,
                                  ft * 128:(ft + 1) * 128],
                            htv[pan][:, 2 * a:2 * a + 2, xhl, :],
                            start=(ii == 0), stop=(ii == 3 * NPAIR - 1),
                            perf_mode=DR)
                        ii += 1

            mem = {}
            # ---- loop A: fk, fv, kv, local scan --------------------------
            for pan in range(NPANEL):
                for ft in range(FT):
                    ps = {}
                    for m, nm in enumerate(("kre", "kim", "vre", "vim")):
                        pt = psp.tile([128, PANEL], F32, tag=f"ps_{nm}", name=f"ps_{nm}")
                        fwd_matmuls(pt, m, pan, ft)
                        ps[nm] = pt
                    s = {}
                    for m, nm in enumerate(("kre", "kim", "vre", "vim")):
                        t = fkp.tile([128, PANEL], BF16, tag=f"s_{nm}", name=f"s_{nm}")
                        nc.scalar.copy(t[:], ps[nm][:])
                        if has_bias:
                            c = m * 4 + ft
                            nc.vector.tensor_scalar_add(
                                t[:], t[:], biasc[:, c:c + 1])
                        s[nm] = t
                    t1 = ttp.tile([128, PANEL], BF16, tag="tt", name="tt")
                    nc.vector.tensor_tensor(t1[:], s["kre"][:], s["vre"][:],
                                            op=AT.mult)
                    t2 = ttp.tile([128, PANEL], BF16, tag="tt", name="tt")
                    nc.vector.tensor_tensor(t2[:], s["kim"][:], s["vim"][:],
                                            op=AT.mult)
                    kvre = kvp.tile([128, PANEL], F32, tag="kvre", name="kvre")
                    nc.gpsimd.tensor_tensor(kvre[:], t1[:], t2[:],
                                            op=AT.subtract)
                    t3 = ttp.tile([128, PANEL], BF16, tag="tt", name="tt")
                    nc.vector.tensor_tensor(t3[:], s["kre"][:], s["vim"][:],
                                            op=AT.mult)
                    t4 = ttp.tile([128, PANEL], BF16, tag="tt", name="tt")
                    nc.vector.tensor_tensor(t4[:], s["kim"][:], s["vre"][:],
                                            op=AT.mult)
                    kvim = kvp.tile([128, PANEL], F32, tag="kvim", name="kvim")
                    nc.gpsimd.tensor_tensor(kvim[:], t3[:], t4[:], op=AT.add)
                    if ft == 0:
                        # row 0 carries (DC, nyquist): plain real products
                        nc.vector.tensor_tensor(
                            kvre[0:1, :], s["kre"][0:1, :], s["vre"][0:1, :],
                            op=AT.mult)
                        nc.vector.tensor_tensor(
                            kvim[0:1, :], s["kim"][0:1, :], s["vim"][0:1, :],
                            op=AT.mult)
                    for ri, kv in (("re", kvre), ("im", kvim)):
                        mt = memp.tile([128, PANEL], BF16,
                                       tag=f"mem_{ri}{ft}_{pan}", name=f"mem_{ri}{ft}_{pan}")
                        init = (0.0 if pan == 0
                                else mem[(pan - 1, ri, ft)][:, PANEL - 1:PANEL])
                        nc.vector.tensor_tensor_scan(
                            mt[:], kv[:], kv[:], init,
                            op0=AT.add, op1=AT.bypass)
                        mem[(pan, ri, ft)] = mt

            # ---- totals exchange (hidden under fq matmuls) ---------------
            tot = cp.tile([128, 8], F32, tag="tot", name="tot")
            for ft in range(FT):
                nc.vector.tensor_copy(
                    tot[:, ft:ft + 1],
                    mem[(NPANEL - 1, "re", ft)][:, PANEL - 1:PANEL])
                nc.vector.tensor_copy(
                    tot[:, 4 + ft:5 + ft],
                    mem[(NPANEL - 1, "im", ft)][:, PANEL - 1:PANEL])
            nc.sync.dma_start(cc_in.ap(), tot[:])
            nc.gpsimd.collective_compute(
                "AllGather", AT.bypass,
                replica_groups=[list(range(NCORES))],
                ins=[cc_in[:].opt()], outs=[cc_out[:].opt()])
            g = cp.tile([128, 64], F32, tag="g", name="g")
            nc.sync.dma_start(
                g[:].rearrange("p (c j) -> p c j", c=NCORES),
                cc_out.ap().rearrange("(c p) j -> p c j", c=NCORES))
            gm = cp.tile([128, 64], F32, tag="gm", name="gm")
            nc.vector.tensor_tensor(gm[:], g[:], mask[:], op=AT.mult)
            gv = gm[:].rearrange("p (c j) -> p c j", c=8)
            s1 = cp.tile([128, 32], F32, tag="s1", name="s1")
            nc.vector.tensor_tensor(
                s1[:].rearrange("p (c j) -> p c j", c=4),
                gv[:, 0:4, :], gv[:, 4:8, :], op=AT.add)
            s1v = s1[:].rearrange("p (c j) -> p c j", c=4)
            s2 = cp.tile([128, 16], F32, tag="s2", name="s2")
            nc.vector.tensor_tensor(
                s2[:].rearrange("p (c j) -> p c j", c=2),
                s1v[:, 0:2, :], s1v[:, 2:4, :], op=AT.add)
            pref = cp.tile([128, 8], F32, tag="pref", name="pref")
            nc.vector.tensor_tensor(pref[:], s2[:, 0:8], s2[:, 8:16],
                                    op=AT.add)

            # ---- loop C: fq, prefix, Z, output matmul, residual ----------
            # fq blocks run two panels ahead of the Z/output blocks so the
            # PE keeps streaming fq matmuls while the AllGather completes.
            def fq_block(pan):
                sq = {}
                for ft in range(FT):
                    for m, nm in ((4, "qre"), (5, "qim")):
                        pt = psp.tile([128, PANEL], F32,
                                      tag=("ps_kre" if nm == "qre"
                                           else "ps_kim"),
                                      name=f"ps_{nm}")
                        fwd_matmuls(pt, m, pan, ft)
                        t = fqp.tile([128, PANEL], BF16, tag=f"s_{nm}{ft}",
                                     name=f"s_{nm}{ft}")
                        nc.scalar.activation(
                            t[:], pt[:], mybir.ActivationFunctionType.Copy,
                            scale=2.0 ** -19)
                        if has_bias:
                            c = m * 4 + ft
                            nc.vector.tensor_scalar_add(
                                t[:], t[:], biasc[:, c:c + 1])
                        sq[(nm, ft)] = t
                return sq

            def zout_block(pan, sq):
                z = {}
                for ft in range(FT):
                    mre = mem[(pan, "re", ft)]
                    mim = mem[(pan, "im", ft)]
                    nc.vector.tensor_scalar_add(mre[:], mre[:],
                                                pref[:, ft:ft + 1])
                    nc.vector.tensor_scalar_add(mim[:], mim[:],
                                                pref[:, 4 + ft:5 + ft])
                    sqre, sqim = sq[("qre", ft)], sq[("qim", ft)]
                    for ri, (a, b_) in (("re", (sqre, sqim)),
                                        ("im", (sqim, sqre))):
                        neg = ri == "re"
                        t1 = ttp.tile([128, PANEL], BF16, tag="tt", name="tt")
                        nc.vector.tensor_tensor(t1[:], mre[:], a[:],
                                                op=AT.mult)
                        t2 = ttp.tile([128, PANEL], BF16, tag="tt", name="tt")
                        nc.vector.tensor_tensor(t2[:], mim[:], b_[:],
                                                op=AT.mult)
                        zt = zp.tile([128, PANEL], BF16, tag=f"z_{ri}{ft}",
                                     name=f"z_{ri}{ft}")
                        eng = nc.gpsimd if neg else nc.vector
                        eng.tensor_tensor(
                            zt[:], t1[:], t2[:],
                            op=(AT.subtract if neg else AT.add))
                        if ft == 0:
                            # row 0 carries (DC, nyq): plain real products
                            nc.vector.tensor_tensor(
                                zt[0:1, :], (mre if neg else mim)[0:1, :],
                                (sqre if neg else sqim)[0:1, :], op=AT.mult)
                        z[(ri, ft)] = zt
                for sub in range(4):
                    r0 = pan * PANEL + sub * 128
                    ob = iop.tile([128, D], BF16, tag="ob", name="ob")
                    nc.sync.dma_start(ob[:], outp_d.ap()[r0:r0 + 128, :])
                    rs = iop.tile([128, D], F32, tag="rs", name="rs")
                    s0, s1c = sub * 128, (sub + 1) * 128
                    for half in range(2):
                        d0, d1 = half * 512, (half + 1) * 512
                        pv = psp.tile([128, 512], F32, tag="ps_vre",
                                      name="ps_pv")
                        for ft in range(FT):
                            nc.tensor.matmul(
                                pv[:], z[("re", ft)][:, s0:s1c],
                                ab[ft][:, d0:d1],
                                start=(ft == 0), stop=False)
                        for ft in range(FT):
                            nc.tensor.matmul(
                                pv[:], z[("im", ft)][:, s0:s1c],
                                ab[4 + ft][:, d0:d1],
                                start=False, stop=(ft == FT - 1))
                        nc.vector.tensor_tensor(rs[:, d0:d1], pv[:],
                                                ob[:, d0:d1], op=AT.add)
                        nc.sync.dma_start(res_d.ap()[r0:r0 + 128, d0:d1],
                                          rs[:, d0:d1])

            sqs = {0: fq_block(0), 1: fq_block(1), 2: fq_block(2)}
            for pan in range(NPANEL):
                zout_block(pan, sqs.pop(pan))
                if pan + 3 < NPANEL:
                    sqs[pan + 3] = fq_block(pan + 3)

    _legalize_waits(nc)
    return nc


def _program(has_bias=False):
    key = ("merged", has_bias)
    if key not in _cache:
        _cache[key] = _build(has_bias)
    return _cache[key]


def kernel(output, hidden_states, Wq, bq, Wk, bk, Wv, bv, gate, _trace=False):
    from concourse import bass_utils

    output = np.asarray(output, dtype=np.float32)
    hidden = np.asarray(hidden_states, dtype=np.float32)
    cst = _host_constants(
        np.asarray(Wq, np.float32), np.asarray(bq, np.float32),
        np.asarray(Wk, np.float32), np.asarray(bk, np.float32),
        np.asarray(Wv, np.float32), np.asarray(bv, np.float32),
        np.asarray(gate, np.float32))
    has_bias = bool(np.any(cst["biasc"]))
    nc = _program(has_bias)

    chunks = [(c // 4, c % 4) for c in range(NCORES)]
    shared = {"wall": cst["wall"], "ab": cst["ab"]}
    if has_bias:
        bc = np.zeros((128, 24), np.float32)
        for m in range(6):
            bc[:, m * 4:(m + 1) * 4] = cst["biasc"][m].reshape(4, 128).T
        shared["biasc"] = bc

    in_maps = []
    for c, (b, j) in enumerate(chunks):
        im = dict(shared)
        ht = np.ascontiguousarray(
            hidden[b, j * CHUNK:(j + 1) * CHUNK, :].T)
        im["htp"] = _pack_ht(ht)
        im["outp"] = output[b, j * CHUNK:(j + 1) * CHUNK, :].astype(BF16NP)
        mask = np.zeros((128, 64), np.float32)
        for c2, (b2, j2) in enumerate(chunks):
            if b2 == b and j2 < j:
                mask[:, c2 * 8:(c2 + 1) * 8] = 1.0
        im["mask"] = mask
        in_maps.append(im)

    res = bass_utils.run_bass_kernel_spmd(
        nc, in_maps, core_ids=list(range(NCORES)), trace=_trace)

    out = np.empty((B, S, D), dtype=np.float32)
    for c, (b, j) in enumerate(chunks):
        out[b, j * CHUNK:(j + 1) * CHUNK, :] = res.results[c]["res"]
    if _trace:
        kernel._last = res
    return out



# revision 3
# speedup vs baseline: 1.0793x; 1.0793x over previous
"""Trainium2 Bass kernel for nn_HRRAdaptedAttention (B=2, S=8192, D=1024).

out = output + gate * irfft(cumsum_s(rfft(k)*rfft(v)) * conj(rfft(q))),
q/k/v = hidden @ W.T + b.

Single merged launch per core (chunk = 2048 positions, (batch, quarter)
per core). The rfft/irfft are folded into the projection weights on the
host; the nyquist row is packed into the (all-zero) sin(f=0) column of
each S-matrix, with 1-row DVE fixups where the complex-multiply formula
would mix the DC and nyquist rows. Projection matmuls run as fp8e4m3
hi+lo splits in DoubleRow perf mode (2 contraction slices/instr at 0.5
cyc/row); the cross-chunk cumsum carry is resolved in-kernel with an
AllGather of per-chunk totals (hidden under the fq matmuls), so there
is no second launch and no kv/ht DRAM round trip.
"""

import numpy as np
import ml_dtypes

B, S, D = 2, 8192, 1024
NCORES = 8
CHUNK = 2048
PANEL = 512
NPANEL = CHUNK // PANEL
FT = 4                    # 128-row freq tiles f=0..511 (nyq packed in im row 0)
NDP = 8                   # 128-row contraction slices of D
NPAIR = NDP // 2

F8NP = ml_dtypes.float8_e4m3
BF16NP = ml_dtypes.bfloat16

_cache = {}


def _split8(x):
    hi = x.astype(F8NP)
    lo = (x - hi.astype(np.float32)).astype(F8NP)
    return hi, lo


def _pack_w(M):
    """[1024, 512] f32 -> [128, dp(8), hl(2), 512] fp8 -> [128, 8192]."""
    hi, lo = _split8(np.asarray(M, np.float32))
    out = np.empty((128, NDP, 2, 512), F8NP)
    out[:, :, 0, :] = hi.reshape(NDP, 128, 512).transpose(1, 0, 2)
    out[:, :, 1, :] = lo.reshape(NDP, 128, 512).transpose(1, 0, 2)
    return out.reshape(128, -1)


def _pack_ht(ht):
    """[1024, 2048] f32 -> [128, pan(4), dp(8), hl(2), pos(512)] fp8."""
    hi, lo = _split8(ht)
    out = np.empty((128, NPANEL, NDP, 2, PANEL), F8NP)
    h4 = hi.reshape(NDP, 128, NPANEL, PANEL)
    l4 = lo.reshape(NDP, 128, NPANEL, PANEL)
    out[:, :, :, 0, :] = h4.transpose(1, 2, 0, 3)
    out[:, :, :, 1, :] = l4.transpose(1, 2, 0, 3)
    return out.reshape(128, -1)


def _host_constants(Wq, bq, Wk, bk, Wv, bv, gate):
    d = np.arange(D, dtype=np.float64)
    f = np.arange(513, dtype=np.float64)
    ang = 2.0 * np.pi * np.outer(d, f) / D
    C = np.cos(ang)
    Sm = -np.sin(ang)

    def fold(W, sign_s=1.0):
        Wt = W.T.astype(np.float64)
        MC = Wt @ C
        MS = sign_s * (Wt @ Sm)
        MS[:, 0] = MC[:, 512]          # nyquist packed into sin(f=0) col
        return MC[:, :512].astype(np.float32), MS[:, :512].astype(np.float32)

    MkC, MkS = fold(Wk)
    MvC, MvS = fold(Wv)
    MqC, MqS = fold(Wq, sign_s=-1.0)   # conj(fq) folded; nyq col has no sign
    # Z = mem*conj(fq) is ~1e6-scale, far outside fp8 range. The fq
    # PSUM->SBUF copy applies 2^-19 (activation scale) so Z comes out
    # pre-scaled; A/B carry the inverse. Pure exponent shift.
    ZSC = 2.0 ** -19

    g = float(np.asarray(gate).reshape(-1)[0])
    w = np.full(513, 2.0)
    w[0] = 1.0
    w[512] = 1.0
    scale = (w * g / (D * ZSC))[:, None]
    A = (scale * C.T).astype(np.float32)       # [513, D]
    Bm = (scale * Sm.T).astype(np.float32)
    Bout = Bm[:512].copy()
    Bout[0] = A[512]                            # nyquist row in B row 0
    Aout = A[:512]

    def foldb(b, sign_s=1.0):
        b64 = np.asarray(b, np.float64)
        bc = b64 @ C
        bs = sign_s * (b64 @ Sm)
        bs[0] = bc[512]
        return bc[:512].astype(np.float32), bs[:512].astype(np.float32)

    bkC, bkS = foldb(bk)
    bvC, bvS = foldb(bv)
    bqC, bqS = foldb(bq, sign_s=-1.0)
    bqC *= ZSC
    bqS *= ZSC
    biasc = np.stack([bkC, bkS, bvC, bvS, bqC, bqS])   # [6, 512]

    wall = np.concatenate(
        [_pack_w(M) for M in (MkC, MkS, MvC, MvS, MqC, MqS)], axis=1)
    ab = np.concatenate([Aout, Bout], axis=0).astype(BF16NP)  # [1024, 1024]
    return dict(wall=wall, ab=ab, biasc=biasc)


_WAIT_EXEMPT = {
    "InstNoOp", "InstEventSemaphore", "InstUnconditionalBranch",
    "InstRegisterMove", "InstCall", "InstISA",
}


def _legalize_waits(nc, max_waits=1):
    """TRN2 instruction structs hold one sync-wait command; move extra waits
    onto same-engine nops inserted just before the instruction."""
    import bass_rust
    import concourse.mybir as mybir
    ctr = 0
    for fn in nc.m.functions:
        for blk in fn.blocks:
            new = []
            for inst in blk.instructions:
                if (type(inst).__name__ not in _WAIT_EXEMPT
                        and inst.sync_info is not None):
                    waits = list(inst.sync_info.on_wait)
                    if len(waits) > max_waits:
                        for w in waits[:-max_waits]:
                            nop = mybir.InstNoOp(
                                name=f"I-lglnop-{ctr}", ins=[], outs=[])
                            ctr += 1
                            nop.engine = inst.engine
                            nop.sync_info = bass_rust.SyncInfo(
                                on_wait=[w], on_update=[])
                            new.append(nop)
                        inst.sync_info = bass_rust.SyncInfo(
                            on_wait=waits[-max_waits:],
                            on_update=inst.sync_info.on_update)
                new.append(inst)
            blk.instructions = new


def _build(has_bias):
    import concourse.bass as bass
    import concourse.mybir as mybir
    import concourse.tile as tile
    F32, F8, BF16 = mybir.dt.float32, mybir.dt.float8e4, mybir.dt.bfloat16
    AT = mybir.AluOpType
    DR = mybir.MatmulPerfMode.DoubleRow

    nc = bass.Bass("TRN2", target_bir_lowering=False, debug=False,
                   num_devices=NCORES)
    htp_d = nc.dram_tensor("htp", [128, NPANEL * 8192], F8,
                           kind="ExternalInput")
    w_d = nc.dram_tensor("wall", [128, 6 * 8192], F8, kind="ExternalInput")
    ab_d = nc.dram_tensor("ab", [1024, 1024], BF16, kind="ExternalInput")
    outp_d = nc.dram_tensor("outp", [CHUNK, D], BF16, kind="ExternalInput")
    mask_d = nc.dram_tensor("mask", [128, 64], F32, kind="ExternalInput")
    if has_bias:
        biasc_d = nc.dram_tensor("biasc", [128, 24], F32,
                                 kind="ExternalInput")
    res_d = nc.dram_tensor("res", [CHUNK, D], F32, kind="ExternalOutput")
    cc_in = nc.dram_tensor("cc_in", [128, 8], F32)
    cc_out = nc.dram_tensor("cc_out", [NCORES * 128, 8], F32)

    with tile.TileContext(nc) as tc:
        with (
            tc.tile_pool(name="wpool", bufs=1) as wp,
            tc.tile_pool(name="htpool", bufs=1) as hp,
            tc.tile_pool(name="const", bufs=1) as cp,
            tc.tile_pool(name="fkv", bufs=2) as fkp,
            tc.tile_pool(name="fq3", bufs=3) as fqp,
            tc.tile_pool(name="tt", bufs=4) as ttp,
            tc.tile_pool(name="kv", bufs=2) as kvp,
            tc.tile_pool(name="mem", bufs=1) as memp,
            tc.tile_pool(name="z", bufs=2) as zp,
            tc.tile_pool(name="io", bufs=2) as iop,
            tc.tile_pool(name="ps", bufs=2, space="PSUM") as psp,
        ):
            # DMA order is tuned so the PE can start ~7us in: hi halves of
            # the k/v weights and panel-0 ht first, then the lo halves.
            w = [wp.tile([128, 8192], F8, tag=f"w{m}", name=f"w{m}") for m in range(6)]
            htp = [hp.tile([128, 8192], F8, tag=f"ht{p}", name=f"ht{p}")
                   for p in range(NPANEL)]
            wvd = w_d.ap().rearrange("p (m dp hl f) -> p m dp hl f",
                                     m=6, dp=NDP, hl=2)
            htd = htp_d.ap().rearrange("p (pan dp hl x) -> p pan dp hl x",
                                       pan=NPANEL, dp=NDP, hl=2)
            wvs = [w[m][:].rearrange("p (dp hl f) -> p dp hl f",
                                     dp=NDP, hl=2) for m in range(6)]
            hts = [htp[p][:].rearrange("p (dp hl x) -> p dp hl x",
                                       dp=NDP, hl=2) for p in range(NPANEL)]

            def load_w(m, hl):
                nc.sync.dma_start(wvs[m][:, :, hl, :], wvd[:, m, :, hl, :])

            def load_ht(p, hl):
                nc.sync.dma_start(hts[p][:, :, hl, :], htd[:, p, :, hl, :])

            load_ht(0, 0)
            load_w(0, 0)
            load_w(0, 1)
            load_ht(0, 1)
            for m in range(1, 4):
                load_w(m, 0)
                load_w(m, 1)
            load_w(4, 0)
            load_w(5, 0)
            load_w(4, 1)
            load_w(5, 1)
            for p in range(1, NPANEL):
                load_ht(p, 0)
                load_ht(p, 1)
            ab = [wp.tile([128, 1024], BF16, tag=f"ab{i}", name=f"ab{i}")
                  for i in range(8)]
            for i in range(8):
                nc.sync.dma_start(ab[i][:], ab_d.ap()[i * 128:(i + 1) * 128, :])
            mask = cp.tile([128, 64], F32, tag="mask", name="mask")
            nc.sync.dma_start(mask[:], mask_d.ap())
            if has_bias:
                biasc = cp.tile([128, 24], F32, tag="biasc", name="biasc")
                nc.sync.dma_start(biasc[:], biasc_d.ap())

            wv, htv = wvs, hts

            def fwd_matmuls(pt, m, pan, ft, combos=3):
                # hh first (hi weights + hi data), then lh (lo weights),
                # then hl (lo data) — matches the DMA arrival order.
                # k/v (m<4) run 2 combos (full-W x hi-X): the dropped W@Xlo
                # term costs ~1e-2 rel err, inside the 2e-2 budget; q keeps
                # all 3 (its error hits Z unaveraged).
                ii = 0
                for (whl, xhl) in ((0, 0), (1, 0), (0, 1))[:combos]:
                    for a in range(NPAIR):
                        nc.tensor.matmul(
                            pt[:],
                            wv[m][:, 2 * a:2 * a + 2, whl,
                                  ft * 128:(ft + 1) * 128],
                            htv[pan][:, 2 * a:2 * a + 2, xhl, :],
                            start=(ii == 0), stop=(ii == combos * NPAIR - 1),
                            perf_mode=DR)
                        ii += 1

            mem = {}
            # ---- loop A: fk, fv, kv, local scan --------------------------
            for pan in range(NPANEL):
                for ft in range(FT):
                    ps = {}
                    for m, nm in enumerate(("kre", "kim", "vre", "vim")):
                        pt = psp.tile([128, PANEL], F32, tag=f"ps_{nm}", name=f"ps_{nm}")
                        fwd_matmuls(pt, m, pan, ft, combos=2)
                        ps[nm] = pt
                    s = {}
                    for m, nm in enumerate(("kre", "kim", "vre", "vim")):
                        t = fkp.tile([128, PANEL], BF16, tag=f"s_{nm}", name=f"s_{nm}")
                        nc.scalar.copy(t[:], ps[nm][:])
                        if has_bias:
                            c = m * 4 + ft
                            nc.vector.tensor_scalar_add(
                                t[:], t[:], biasc[:, c:c + 1])
                        s[nm] = t
                    t1 = ttp.tile([128, PANEL], BF16, tag="tt", name="tt")
                    nc.vector.tensor_tensor(t1[:], s["kre"][:], s["vre"][:],
                                            op=AT.mult)
                    t2 = ttp.tile([128, PANEL], BF16, tag="tt", name="tt")
                    nc.vector.tensor_tensor(t2[:], s["kim"][:], s["vim"][:],
                                            op=AT.mult)
                    kvre = kvp.tile([128, PANEL], F32, tag="kvre", name="kvre")
                    nc.gpsimd.tensor_tensor(kvre[:], t1[:], t2[:],
                                            op=AT.subtract)
                    t3 = ttp.tile([128, PANEL], BF16, tag="tt", name="tt")
                    nc.vector.tensor_tensor(t3[:], s["kre"][:], s["vim"][:],
                                            op=AT.mult)
                    t4 = ttp.tile([128, PANEL], BF16, tag="tt", name="tt")
                    nc.vector.tensor_tensor(t4[:], s["kim"][:], s["vre"][:],
                                            op=AT.mult)
                    kvim = kvp.tile([128, PANEL], F32, tag="kvim", name="kvim")
                    nc.gpsimd.tensor_tensor(kvim[:], t3[:], t4[:], op=AT.add)
                    if ft == 0:
                        # row 0 carries (DC, nyquist): plain real products
                        nc.vector.tensor_tensor(
                            kvre[0:1, :], s["kre"][0:1, :], s["vre"][0:1, :],
                            op=AT.mult)
                        nc.vector.tensor_tensor(
                            kvim[0:1, :], s["kim"][0:1, :], s["vim"][0:1, :],
                            op=AT.mult)
                    for ri, kv in (("re", kvre), ("im", kvim)):
                        mt = memp.tile([128, PANEL], BF16,
                                       tag=f"mem_{ri}{ft}_{pan}", name=f"mem_{ri}{ft}_{pan}")
                        init = (0.0 if pan == 0
                                else mem[(pan - 1, ri, ft)][:, PANEL - 1:PANEL])
                        nc.vector.tensor_tensor_scan(
                            mt[:], kv[:], kv[:], init,
                            op0=AT.add, op1=AT.bypass)
                        mem[(pan, ri, ft)] = mt

            # ---- totals exchange (hidden under fq matmuls) ---------------
            tot = cp.tile([128, 8], F32, tag="tot", name="tot")
            for ft in range(FT):
                nc.vector.tensor_copy(
                    tot[:, ft:ft + 1],
                    mem[(NPANEL - 1, "re", ft)][:, PANEL - 1:PANEL])
                nc.vector.tensor_copy(
                    tot[:, 4 + ft:5 + ft],
                    mem[(NPANEL - 1, "im", ft)][:, PANEL - 1:PANEL])
            nc.sync.dma_start(cc_in.ap(), tot[:])
            nc.gpsimd.collective_compute(
                "AllGather", AT.bypass,
                replica_groups=[list(range(NCORES))],
                ins=[cc_in[:].opt()], outs=[cc_out[:].opt()])
            g = cp.tile([128, 64], F32, tag="g", name="g")
            nc.sync.dma_start(
                g[:].rearrange("p (c j) -> p c j", c=NCORES),
                cc_out.ap().rearrange("(c p) j -> p c j", c=NCORES))
            gm = cp.tile([128, 64], F32, tag="gm", name="gm")
            nc.vector.tensor_tensor(gm[:], g[:], mask[:], op=AT.mult)
            gv = gm[:].rearrange("p (c j) -> p c j", c=8)
            s1 = cp.tile([128, 32], F32, tag="s1", name="s1")
            nc.vector.tensor_tensor(
                s1[:].rearrange("p (c j) -> p c j", c=4),
                gv[:, 0:4, :], gv[:, 4:8, :], op=AT.add)
            s1v = s1[:].rearrange("p (c j) -> p c j", c=4)
            s2 = cp.tile([128, 16], F32, tag="s2", name="s2")
            nc.vector.tensor_tensor(
                s2[:].rearrange("p (c j) -> p c j", c=2),
                s1v[:, 0:2, :], s1v[:, 2:4, :], op=AT.add)
            pref = cp.tile([128, 8], F32, tag="pref", name="pref")
            nc.vector.tensor_tensor(pref[:], s2[:, 0:8], s2[:, 8:16],
                                    op=AT.add)

            # ---- loop C: fq, prefix, Z, output matmul, residual ----------
            # fq blocks run two panels ahead of the Z/output blocks so the
            # PE keeps streaming fq matmuls while the AllGather completes.
            def fq_block(pan):
                sq = {}
                for ft in range(FT):
                    for m, nm in ((4, "qre"), (5, "qim")):
                        pt = psp.tile([128, PANEL], F32,
                                      tag=("ps_kre" if nm == "qre"
                                           else "ps_kim"),
                                      name=f"ps_{nm}")
                        fwd_matmuls(pt, m, pan, ft)
                        t = fqp.tile([128, PANEL], BF16, tag=f"s_{nm}{ft}",
                                     name=f"s_{nm}{ft}")
                        nc.scalar.activation(
                            t[:], pt[:], mybir.ActivationFunctionType.Copy,
                            scale=2.0 ** -19)
                        if has_bias:
                            c = m * 4 + ft
                            nc.vector.tensor_scalar_add(
                                t[:], t[:], biasc[:, c:c + 1])
                        sq[(nm, ft)] = t
                return sq

            def zout_block(pan, sq):
                z = {}
                for ft in range(FT):
                    mre = mem[(pan, "re", ft)]
                    mim = mem[(pan, "im", ft)]
                    nc.vector.tensor_scalar_add(mre[:], mre[:],
                                                pref[:, ft:ft + 1])
                    nc.vector.tensor_scalar_add(mim[:], mim[:],
                                                pref[:, 4 + ft:5 + ft])
                    sqre, sqim = sq[("qre", ft)], sq[("qim", ft)]
                    for ri, (a, b_) in (("re", (sqre, sqim)),
                                        ("im", (sqim, sqre))):
                        neg = ri == "re"
                        t1 = ttp.tile([128, PANEL], BF16, tag="tt", name="tt")
                        nc.vector.tensor_tensor(t1[:], mre[:], a[:],
                                                op=AT.mult)
                        t2 = ttp.tile([128, PANEL], BF16, tag="tt", name="tt")
                        nc.vector.tensor_tensor(t2[:], mim[:], b_[:],
                                                op=AT.mult)
                        zt = zp.tile([128, PANEL], BF16, tag=f"z_{ri}{ft}",
                                     name=f"z_{ri}{ft}")
                        eng = nc.gpsimd if neg else nc.vector
                        eng.tensor_tensor(
                            zt[:], t1[:], t2[:],
                            op=(AT.subtract if neg else AT.add))
                        if ft == 0:
                            # row 0 carries (DC, nyq): plain real products
                            nc.vector.tensor_tensor(
                                zt[0:1, :], (mre if neg else mim)[0:1, :],
                                (sqre if neg else sqim)[0:1, :], op=AT.mult)
                        z[(ri, ft)] = zt
                for sub in range(4):
                    r0 = pan * PANEL + sub * 128
                    ob = iop.tile([128, D], BF16, tag="ob", name="ob")
                    nc.sync.dma_start(ob[:], outp_d.ap()[r0:r0 + 128, :])
                    rs = iop.tile([128, D], F32, tag="rs", name="rs")
                    s0, s1c = sub * 128, (sub + 1) * 128
                    for half in range(2):
                        d0, d1 = half * 512, (half + 1) * 512
                        pv = psp.tile([128, 512], F32, tag="ps_vre",
                                      name="ps_pv")
                        for ft in range(FT):
                            nc.tensor.matmul(
                                pv[:], z[("re", ft)][:, s0:s1c],
                                ab[ft][:, d0:d1],
                                start=(ft == 0), stop=False)
                        for ft in range(FT):
                            nc.tensor.matmul(
                                pv[:], z[("im", ft)][:, s0:s1c],
                                ab[4 + ft][:, d0:d1],
                                start=False, stop=(ft == FT - 1))
                        nc.vector.tensor_tensor(rs[:, d0:d1], pv[:],
                                                ob[:, d0:d1], op=AT.add)
                        nc.sync.dma_start(res_d.ap()[r0:r0 + 128, d0:d1],
                                          rs[:, d0:d1])

            sqs = {0: fq_block(0), 1: fq_block(1), 2: fq_block(2)}
            for pan in range(NPANEL):
                zout_block(pan, sqs.pop(pan))
                if pan + 3 < NPANEL:
                    sqs[pan + 3] = fq_block(pan + 3)

    _legalize_waits(nc)
    return nc


def _program(has_bias=False):
    key = ("merged", has_bias)
    if key not in _cache:
        _cache[key] = _build(has_bias)
    return _cache[key]


def kernel(output, hidden_states, Wq, bq, Wk, bk, Wv, bv, gate, _trace=False):
    from concourse import bass_utils

    output = np.asarray(output, dtype=np.float32)
    hidden = np.asarray(hidden_states, dtype=np.float32)
    cst = _host_constants(
        np.asarray(Wq, np.float32), np.asarray(bq, np.float32),
        np.asarray(Wk, np.float32), np.asarray(bk, np.float32),
        np.asarray(Wv, np.float32), np.asarray(bv, np.float32),
        np.asarray(gate, np.float32))
    has_bias = bool(np.any(cst["biasc"]))
    nc = _program(has_bias)

    chunks = [(c // 4, c % 4) for c in range(NCORES)]
    shared = {"wall": cst["wall"], "ab": cst["ab"]}
    if has_bias:
        bc = np.zeros((128, 24), np.float32)
        for m in range(6):
            bc[:, m * 4:(m + 1) * 4] = cst["biasc"][m].reshape(4, 128).T
        shared["biasc"] = bc

    in_maps = []
    for c, (b, j) in enumerate(chunks):
        im = dict(shared)
        ht = np.ascontiguousarray(
            hidden[b, j * CHUNK:(j + 1) * CHUNK, :].T)
        im["htp"] = _pack_ht(ht)
        im["outp"] = output[b, j * CHUNK:(j + 1) * CHUNK, :].astype(BF16NP)
        mask = np.zeros((128, 64), np.float32)
        for c2, (b2, j2) in enumerate(chunks):
            if b2 == b and j2 < j:
                mask[:, c2 * 8:(c2 + 1) * 8] = 1.0
        im["mask"] = mask
        in_maps.append(im)

    res = bass_utils.run_bass_kernel_spmd(
        nc, in_maps, core_ids=list(range(NCORES)), trace=_trace)

    out = np.empty((B, S, D), dtype=np.float32)
    for c, (b, j) in enumerate(chunks):
        out[b, j * CHUNK:(j + 1) * CHUNK, :] = res.results[c]["res"]
    if _trace:
        kernel._last = res
    return out

